# revision 23
# baseline (speedup 1.0000x reference)
"""Trainium2 Bass kernel for causal self-attention + out-proj + residual + LayerNorm.

v3: heads (tensor-parallel) across 8 cores for QKV+attention (kernel A),
then sequence-parallel across 8 cores for out-proj + residual + LN (kernel B).

Design:
- x is transposed + cast to bf16 on the HOST (free): no on-device transposes.
  Chunk-contiguous layout so each DMA is 8 KB/partition contiguous.
- All matmul operands bf16 (FWL weight loads, half SBUF/DMA traffic).
- QKV projection is chunk-pipelined with attention (hides under the exp wall).
- Causal trimming: diagonal-chunk score/PV matmuls and exps only touch the
  valid column range.
- exp split across Scalar (ACT true exp) and Vector (DVE Schraudolph bit-trick
  exp -> bf16 bits via int16 output): the two engines run concurrently.
- V computed directly in [token, dim] layout (stationary = xT tile), softmax
  denominator via an appended ones column.
- pv PSUM released early via ACT-engine copy to SBUF; normalize mul and
  diagonal mask mul run on the otherwise-idle GPSIMD engine.
"""

import math
from contextlib import ExitStack

import numpy as np
import ml_dtypes

import concourse.bass as bass
import concourse.tile as tile
from concourse import bacc, mybir
from concourse.bass_utils import run_bass_kernel_spmd

BF16NP = ml_dtypes.bfloat16


# NTFF-trace shim: make run_bass_kernel_spmd(trace=True) usable in containers
# whose antenv lacks axon_hooks (harmless when tracing is off).
def _install_trace_shim():
    import sys, types
    try:
        import antenv.axon_hooks  # noqa: F401
        return
    except ImportError:
        pass
    try:
        import antenv
        from trn_agent_boot.trn_boot import _ntff_profile_via_ctypes
        hook = _ntff_profile_via_ctypes("/opt/axon/libaxon_pjrt.so")
        mod = types.ModuleType("antenv.axon_hooks")
        mod.get_axon_ntff_profile_hook = lambda: hook
        mod.set_axon_ntff_profile_hook = lambda h: None
        sys.modules["antenv.axon_hooks"] = mod
        antenv.axon_hooks = mod
        import concourse.bass_utils as _bu
        _bu.upload_artifacts = lambda tmpdir: "local://skipped"
    except Exception:
        pass


_install_trace_shim()

F32 = mybir.dt.float32
BF16 = mybir.dt.bfloat16
I16 = mybir.dt.int16
EXP = mybir.ActivationFunctionType.Exp
COPYF = mybir.ActivationFunctionType.Copy
SQRT = mybir.ActivationFunctionType.Sqrt
ADD = mybir.AluOpType.add
MULT = mybir.AluOpType.mult
SUB = mybir.AluOpType.subtract

T_FULL = 4096
D = 1024
HEADS = 16
NCORES = 8
LN_EPS = 1e-5

# Schraudolph exp -> bf16 bit pattern via int16: exp(x) ~= bf16_bits(int16(x*A16 + B16))
A16 = 128.0 / math.log(2.0)
B16 = 16251.0  # tuned for truncation toward zero on positive values

_CACHE = {}
LAST_RESULTS = {}


def build_kernel_a(T=T_FULL):
    """Per core: 2 heads. Computes at = softmax(QK^T/sqrt(d)) @ V in layout
    [128 = 2*64 head dims, T], bf16, normalized."""
    nc = bacc.Bacc("TRN2", target_bir_lowering=False, debug=False)
    KD = D // 128          # 8 contraction tiles over D
    NQ = T // 512          # chunks of 512 tokens

    # wpack: wq|wk|wv stationary tiles + trimask, single bf16 DMA
    wp_d = nc.dram_tensor("wpack", [128, 3, KD, 128], BF16, kind="ExternalInput")
    tm_d = nc.dram_tensor("trimask", [128, 128], BF16, kind="ExternalInput")
    # bpack: bq | bk | bvb  (f32)
    bp_d = nc.dram_tensor("bpack", [128, 130], F32, kind="ExternalInput")
    xt_d = nc.dram_tensor("xt", [128, NQ, KD, 512], BF16, kind="ExternalInput")
    at_d = nc.dram_tensor("at_out", [128, T], BF16, kind="ExternalOutput")

    with tile.TileContext(nc) as tc, ExitStack() as ctx:
        const = ctx.enter_context(tc.tile_pool(name="const", bufs=1))
        persist = ctx.enter_context(tc.tile_pool(name="persist", bufs=1))
        xtp = ctx.enter_context(tc.tile_pool(name="xtp", bufs=2))
        e_pool = ctx.enter_context(tc.tile_pool(name="e_pool", bufs=4))
        rb_pool = ctx.enter_context(tc.tile_pool(name="rb_pool", bufs=2))
        qkv_ps = ctx.enter_context(tc.tile_pool(name="qkv_ps", bufs=2, space="PSUM"))
        s_ps = ctx.enter_context(tc.tile_pool(name="s_ps", bufs=2, space="PSUM"))
        pv_ps = ctx.enter_context(tc.tile_pool(name="pv_ps", bufs=1, space="PSUM"))

        wp_sb = const.tile([128, 3, KD, 128], BF16, tag="wp")
        nc.sync.dma_start(wp_sb[:], wp_d.ap())
        wq_sb, wk_sb, wv_sb = wp_sb[:, 0], wp_sb[:, 1], wp_sb[:, 2]

        # first data chunk right behind the weights
        xt_tiles = []
        xt_c0 = xtp.tile([128, KD, 512], BF16, tag="xt", name="xt_0")
        nc.sync.dma_start(xt_c0[:], xt_d.ap()[:, 0])
        xt_tiles.append(xt_c0)

        bp_sb = const.tile([128, 130], F32, tag="bp")
        nc.sync.dma_start(bp_sb[:], bp_d.ap())
        bq_sb, bk_sb, bvb_sb = bp_sb[:, 0:1], bp_sb[:, 1:2], bp_sb[:, 2:130]
        trimask = const.tile([128, 128], BF16, tag="tm")
        nc.sync.dma_start(trimask[:], tm_d.ap())

        qt_sb = persist.tile([128, T], BF16, tag="qt")
        kt_sb = persist.tile([128, T], BF16, tag="kt")
        # V natural layout per 128-token tile: 64 V cols + ones + zero, per head
        v_sb = persist.tile([128, T // 128, 132], BF16, tag="v")
        nc.gpsimd.memset(v_sb[:, :, 64:65], 1.0)
        nc.gpsimd.memset(v_sb[:, :, 65:66], 0.0)
        nc.gpsimd.memset(v_sb[:, :, 130:131], 1.0)
        nc.gpsimd.memset(v_sb[:, :, 131:132], 0.0)
        at_sb = persist.tile([128, T], BF16, tag="at")

        def do_qkv(c):
            c_sl = slice(c * 512, (c + 1) * 512)
            if c < len(xt_tiles):
                xt_c = xt_tiles[c]
            else:
                xt_c = xtp.tile([128, KD, 512], BF16, tag="xt", name=f"xt_{c}")
                nc.sync.dma_start(xt_c[:], xt_d.ap()[:, c])
            # prefetch next chunk
            if c + 1 == len(xt_tiles) and c + 1 < NQ:
                xt_n = xtp.tile([128, KD, 512], BF16, tag="xt", name=f"xt_{c + 1}")
                nc.sync.dma_start(xt_n[:], xt_d.ap()[:, c + 1])
                xt_tiles.append(xt_n)

            for w_sb, b_sb, dst in ((wq_sb, bq_sb, qt_sb), (wk_sb, bk_sb, kt_sb)):
                pp = qkv_ps.tile([128, 512], F32, tag="pp", name=f"pp_{c}_{dst.name}")
                for kt in range(KD):
                    nc.tensor.matmul(pp[:], w_sb[:, kt, :], xt_c[:, kt, :],
                                     start=(kt == 0), stop=(kt == KD - 1))
                nc.vector.tensor_scalar(out=dst[:, c_sl], in0=pp[:],
                                        scalar1=b_sb, scalar2=None, op0=ADD)
            for tt in range(4):
                t_tile = c * 4 + tt
                vp = qkv_ps.tile([128, 128], F32, tag="pp", name=f"vp_{t_tile}")
                for kt in range(KD):
                    nc.tensor.matmul(vp[:], xt_c[:, kt, tt * 128:(tt + 1) * 128],
                                     wv_sb[:, kt, :],
                                     start=(kt == 0), stop=(kt == KD - 1))
                dst = v_sb[:, t_tile, :].rearrange("p (a b) -> p a b", a=2)[:, :, 0:64]
                nc.vector.tensor_tensor(
                    out=dst, in0=vp[:].rearrange("p (a b) -> p a b", a=2),
                    in1=bvb_sb.rearrange("p (a b) -> p a b", a=2),
                    op=ADD)

        def do_attention(c):
            c_sl = slice(c * 512, (c + 1) * 512)
            nkt = 4 * (c + 1)
            pv = [pv_ps.tile([66, 512], F32, tag=f"pv{h}", name=f"pv{h}_{c}")
                  for h in (0, 1)]

            def emit_pv(kt, esb, o):
                for h in (0, 1):
                    nc.tensor.matmul(pv[h][:, o:512],
                                     v_sb[:, kt, 66 * h:66 * h + 66],
                                     esb[:, h, o:512],
                                     start=(kt == 0), stop=(kt == nkt - 1),
                                     skip_group_check=True)

            prev = None
            prev_o = 0
            for kt in range(nkt):
                o = max(0, kt * 128 - c * 512)
                diag = kt >= nkt - 4
                sp = s_ps.tile([128, 2, 512], F32, tag="s", name=f"s_{c}_{kt}")
                for h in (0, 1):
                    h_sl = slice(64 * h, 64 * h + 64)
                    nc.tensor.matmul(sp[:, h, o:512],
                                     kt_sb[h_sl, kt * 128:(kt + 1) * 128],
                                     qt_sb[h_sl, c * 512 + o:(c + 1) * 512],
                                     start=True, stop=True)
                esb = e_pool.tile([128, 2, 512], BF16, tag="e", name=f"e_{c}_{kt}")
                if not diag and (kt % 2 == 1):
                    # Schraudolph exp on DVE: bf16 bits via int16 output
                    nc.vector.tensor_scalar(out=esb[:].bitcast(I16), in0=sp[:],
                                            scalar1=A16, scalar2=B16,
                                            op0=MULT, op1=ADD)
                elif not diag:
                    nc.scalar.activation(out=esb[:], in_=sp[:], func=EXP)
                else:
                    if o == 0:
                        nc.scalar.activation(out=esb[:], in_=sp[:], func=EXP)
                    else:
                        for h in (0, 1):
                            nc.scalar.activation(out=esb[:, h, o:512],
                                                 in_=sp[:, h, o:512], func=EXP)
                    for h in (0, 1):
                        nc.vector.tensor_mul(esb[:, h, o:o + 128],
                                             esb[:, h, o:o + 128], trimask[:])
                if prev is not None:
                    emit_pv(kt - 1, prev, prev_o)
                prev, prev_o = esb, o
            emit_pv(nkt - 1, prev, prev_o)

            # epilogue: denominator broadcast + reciprocal + normalize
            for h in (0, 1):
                r1 = rb_pool.tile([1, 512], F32, tag="r1", name=f"r1{h}_{c}")
                nc.vector.tensor_copy(r1[:], pv[h][64:65, :])
                rb = rb_pool.tile([128, 512], F32, tag="rb", name=f"rb{h}_{c}")
                nc.gpsimd.partition_broadcast(rb[:], r1[:], channels=128)
                nc.vector.reciprocal_approx_fast(out=rb[:], in_=rb[:])
                nc.vector.tensor_mul(at_sb[64 * h:64 * h + 64, c_sl],
                                     pv[h][0:64, :], rb[64 * h:64 * h + 64, :])
            nc.sync.dma_start(at_d.ap()[:, c_sl], at_sb[:, c_sl])

        # software pipeline: QKV runs one chunk ahead of attention so scores
        # never wait on freshly written Q/K/V.
        for c in range(NQ):
            do_qkv(c)
            if c >= 1:
                do_attention(c - 1)
        do_attention(NQ - 1)

    nc.compile()
    return nc


def build_kernel_b(T=T_FULL, ln_affine=False):
    """Per core: slice of T/8 tokens: out-proj + residual(+bout folded on host
    into xb) + LayerNorm (gamma/beta applied only if ln_affine)."""
    nc = bacc.Bacc("TRN2", target_bir_lowering=False, debug=False)
    Tc = T // NCORES
    KD = D // 128
    IDENT = mybir.ActivationFunctionType.Identity

    at_d = nc.dram_tensor("at", [128, KD, Tc], BF16, kind="ExternalInput")
    wo_d = nc.dram_tensor("wout", [128, 2, KD, 512], BF16, kind="ExternalInput")
    xb_d = nc.dram_tensor("xb", [Tc, D], F32, kind="ExternalInput")
    g_d = nc.dram_tensor("gamma", [128, D], F32, kind="ExternalInput")
    be_d = nc.dram_tensor("beta", [128, D], F32, kind="ExternalInput")
    y_d = nc.dram_tensor("y", [Tc, D], F32, kind="ExternalOutput")

    with tile.TileContext(nc) as tc, ExitStack() as ctx:
        const = ctx.enter_context(tc.tile_pool(name="const", bufs=1))
        work = ctx.enter_context(tc.tile_pool(name="work", bufs=2))
        stats = ctx.enter_context(tc.tile_pool(name="stats", bufs=4))
        ps = ctx.enter_context(tc.tile_pool(name="ps", bufs=4, space="PSUM"))

        # interleave at / wout-half0 DMAs so the first matmuls start early
        at_sb = const.tile([128, KD, Tc], BF16, tag="at")
        wo_sb = const.tile([128, 2, KD, 512], BF16, tag="wo")
        nc.sync.dma_start(at_sb[:, 0:4], at_d.ap()[:, 0:4])
        nc.sync.dma_start(wo_sb[:, 0, 0:4], wo_d.ap()[:, 0, 0:4])
        nc.sync.dma_start(at_sb[:, 4:8], at_d.ap()[:, 4:8])
        nc.sync.dma_start(wo_sb[:, 0, 4:8], wo_d.ap()[:, 0, 4:8])
        xb_tiles = []
        for tt in range(Tc // 128):
            xb_t = work.tile([128, D], F32, tag="xb", name=f"xb_{tt}")
            nc.sync.dma_start(xb_t[:], xb_d.ap()[tt * 128:(tt + 1) * 128, :])
            xb_tiles.append(xb_t)
            if tt == 0:
                nc.sync.dma_start(wo_sb[:, 1], wo_d.ap()[:, 1])
        if ln_affine:
            gam_b = const.tile([128, D], F32, tag="gam")
            bet_b = const.tile([128, D], F32, tag="bet")
            nc.sync.dma_start(gam_b[:], g_d.ap())
            nc.sync.dma_start(bet_b[:], be_d.ap())
        eps_sb = const.tile([128, 1], F32, tag="eps")
        nc.vector.memset(eps_sb[:], LN_EPS)

        for tt in range(Tc // 128):
            t_sl = slice(tt * 128, (tt + 1) * 128)
            xb_t = xb_tiles[tt]
            y_t = work.tile([128, D], F32, tag="y")
            for j in (0, 1):
                pp = ps.tile([128, 512], F32, tag="pp")
                for kt in range(KD):
                    nc.tensor.matmul(pp[:], at_sb[:, kt, t_sl],
                                     wo_sb[:, j, kt, :],
                                     start=(kt == 0), stop=(kt == KD - 1))
                nc.vector.tensor_add(y_t[:, j * 512:(j + 1) * 512], pp[:],
                                     xb_t[:, j * 512:(j + 1) * 512])
            st = stats.tile([128, 2, 6], F32, tag="st")
            nc.vector.bn_stats(st[:, 0, :], y_t[:, 0:512])
            nc.vector.bn_stats(st[:, 1, :], y_t[:, 512:1024])
            mv = stats.tile([128, 2], F32, tag="mv")
            nc.vector.bn_aggr(mv[:], st[:])
            sq = stats.tile([128, 1], F32, tag="sq")
            nc.scalar.activation(out=sq[:], in_=mv[:, 1:2], func=SQRT,
                                 bias=eps_sb[:], scale=1.0)
            rstd = stats.tile([128, 1], F32, tag="rstd")
            nc.vector.reciprocal(rstd[:], sq[:])
            # nm = -mu * rstd;  y = y * rstd + nm   (one ACT op)
            nm = stats.tile([128, 1], F32, tag="nm")
            nc.vector.tensor_scalar(out=nm[:], in0=mv[:, 0:1],
                                    scalar1=rstd[:], scalar2=-1.0,
                                    op0=MULT, op1=MULT)
            nc.scalar.activation(out=y_t[:], in_=y_t[:], func=IDENT,
                                 bias=nm[:], scale=rstd[:])
            if ln_affine:
                nc.vector.tensor_mul(y_t[:], y_t[:], gam_b[:])
                nc.vector.tensor_add(y_t[:], y_t[:], bet_b[:])
            nc.sync.dma_start(y_d.ap()[t_sl, :], y_t[:])

    nc.compile()
    return nc


def _get_kernels(T=T_FULL, ln_affine=False):
    key = (T, ln_affine)
    if key not in _CACHE:
        _CACHE[key] = (build_kernel_a(T), build_kernel_b(T, ln_affine))
    return _CACHE[key]


def kernel(x, Wqkv, bqkv, Wout, bout, gamma, beta):
    x = np.asarray(x, dtype=np.float32)
    Wqkv = np.asarray(Wqkv, dtype=np.float32)
    bqkv = np.asarray(bqkv, dtype=np.float32)
    Wout = np.asarray(Wout, dtype=np.float32)
    bout = np.asarray(bout, dtype=np.float32)
    gamma = np.asarray(gamma, dtype=np.float32)
    beta = np.asarray(beta, dtype=np.float32)

    B, T, D_ = x.shape
    assert B == 1 and D_ == D
    d = D // HEADS
    scale = d ** -0.5
    x2d = np.ascontiguousarray(x[0])
    KD = D // 128
    NQ = T // 512

    # host-side layout prep (free): xt[p, c, k, j] = x[c*512+j, k*128+p]
    xt = np.ascontiguousarray(
        x2d.T.reshape(KD, 128, NQ, 512).transpose(1, 2, 0, 3)).astype(BF16NP)
    trimask = np.triu(np.ones((128, 128), np.float32)).astype(BF16NP)

    ln_affine = not (np.all(gamma == 1.0) and np.all(beta == 0.0))
    nc_a, nc_b = _get_kernels(T, ln_affine)

    in_maps_a = []
    for c in range(NCORES):
        r = slice(c * 128, (c + 1) * 128)
        wq = Wqkv[0 * D:1 * D][r]            # [128, D]
        wk = Wqkv[1 * D:2 * D][r] * scale
        wv = Wqkv[2 * D:3 * D][r]
        bv = bqkv[2 * D:3 * D][r]
        # stationary layout [128 part=D-slice, kt, 128 out]
        wpack = np.stack([
            w.T.reshape(KD, 128, 128).transpose(1, 0, 2)
            for w in (wq, wk, wv)], axis=1)  # [128, 3, KD, 128]
        bpack = np.concatenate([
            bqkv[0 * D:1 * D][r].reshape(128, 1),
            (bqkv[1 * D:2 * D][r] * scale).reshape(128, 1),
            np.tile(bv.reshape(1, 128), (128, 1)),
        ], axis=1)  # [128, 130]
        in_maps_a.append({
            "xt": xt,
            "trimask": trimask,
            "wpack": np.ascontiguousarray(wpack).astype(BF16NP),
            "bpack": np.ascontiguousarray(bpack),
        })
    res_a = run_bass_kernel_spmd(nc_a, in_maps_a, core_ids=list(range(NCORES)))
    LAST_RESULTS["a"] = res_a
    at_full = np.concatenate([res_a.results[c]["at_out"] for c in range(NCORES)],
                             axis=0)  # [D, T] bf16

    Tc = T // NCORES
    wout_st = np.ascontiguousarray(
        Wout.T.reshape(KD, 128, 2, 512).transpose(1, 2, 0, 3)).astype(BF16NP)
    gam_rep = np.ascontiguousarray(np.tile(gamma.reshape(1, D), (128, 1)))
    bet_rep = np.ascontiguousarray(np.tile(beta.reshape(1, D), (128, 1)))
    in_maps_b = []
    for c in range(NCORES):
        t_sl = slice(c * Tc, (c + 1) * Tc)
        at_c = at_full[:, t_sl]  # [D, Tc] bf16
        in_maps_b.append({
            "at": np.ascontiguousarray(at_c.reshape(KD, 128, Tc).transpose(1, 0, 2)),
            "wout": wout_st,
            "xb": np.ascontiguousarray(x2d[t_sl] + bout[None, :]),
            "gamma": gam_rep,
            "beta": bet_rep,
        })
    res_b = run_bass_kernel_spmd(nc_b, in_maps_b, core_ids=list(range(NCORES)))
    LAST_RESULTS["b"] = res_b
    y = np.concatenate([res_b.results[c]["y"] for c in range(NCORES)], axis=0)
    return y.reshape(1, T, D).astype(np.float32)


# revision 29
# speedup vs baseline: 1.0235x; 1.0235x over previous
"""Trainium2 Bass kernel for causal self-attention + out-proj + residual + LayerNorm.

v3: heads (tensor-parallel) across 8 cores for QKV+attention (kernel A),
then sequence-parallel across 8 cores for out-proj + residual + LN (kernel B).

Design:
- x is transposed + cast to bf16 on the HOST (free): no on-device transposes.
  Chunk-contiguous layout so each DMA is 8 KB/partition contiguous.
- All matmul operands bf16 (FWL weight loads, half SBUF/DMA traffic).
- QKV projection is chunk-pipelined with attention (hides under the exp wall).
- Causal trimming: diagonal-chunk score/PV matmuls and exps only touch the
  valid column range.
- exp split across Scalar (ACT true exp) and Vector (DVE Schraudolph bit-trick
  exp -> bf16 bits via int16 output): the two engines run concurrently.
- V computed directly in [token, dim] layout (stationary = xT tile), softmax
  denominator via an appended ones column.
- pv PSUM released early via ACT-engine copy to SBUF; normalize mul and
  diagonal mask mul run on the otherwise-idle GPSIMD engine.
"""

import math
from contextlib import ExitStack

import numpy as np
import ml_dtypes

import concourse.bass as bass
import concourse.tile as tile
from concourse import bacc, mybir
from concourse.bass_utils import run_bass_kernel_spmd

BF16NP = ml_dtypes.bfloat16


# NTFF-trace shim: make run_bass_kernel_spmd(trace=True) usable in containers
# whose antenv lacks axon_hooks (harmless when tracing is off).
def _install_trace_shim():
    import sys, types
    try:
        import antenv.axon_hooks  # noqa: F401
        return
    except ImportError:
        pass
    try:
        import antenv
        from trn_agent_boot.trn_boot import _ntff_profile_via_ctypes
        hook = _ntff_profile_via_ctypes("/opt/axon/libaxon_pjrt.so")
        mod = types.ModuleType("antenv.axon_hooks")
        mod.get_axon_ntff_profile_hook = lambda: hook
        mod.set_axon_ntff_profile_hook = lambda h: None
        sys.modules["antenv.axon_hooks"] = mod
        antenv.axon_hooks = mod
        import concourse.bass_utils as _bu
        _bu.upload_artifacts = lambda tmpdir: "local://skipped"
    except Exception:
        pass


_install_trace_shim()

F32 = mybir.dt.float32
BF16 = mybir.dt.bfloat16
I16 = mybir.dt.int16
EXP = mybir.ActivationFunctionType.Exp
COPYF = mybir.ActivationFunctionType.Copy
SQRT = mybir.ActivationFunctionType.Sqrt
ADD = mybir.AluOpType.add
MULT = mybir.AluOpType.mult
SUB = mybir.AluOpType.subtract

T_FULL = 4096
D = 1024
HEADS = 16
NCORES = 8
LN_EPS = 1e-5

# Schraudolph exp -> bf16 bit pattern via int16: exp(x) ~= bf16_bits(int16(x*A16 + B16))
A16 = 128.0 / math.log(2.0)
B16 = 16251.0  # tuned for truncation toward zero on positive values

_CACHE = {}
LAST_RESULTS = {}


def build_kernel_a(T=T_FULL):
    """Per core: 2 heads. Computes at = softmax(QK^T/sqrt(d)) @ V in layout
    [128 = 2*64 head dims, T], bf16, normalized."""
    nc = bacc.Bacc("TRN2", target_bir_lowering=False, debug=False)
    KD = D // 128          # 8 contraction tiles over D
    NQ = T // 512          # chunks of 512 tokens

    # wpack: wq|wk|wv stationary tiles + trimask, single bf16 DMA
    wp_d = nc.dram_tensor("wpack", [128, 3, KD, 128], BF16, kind="ExternalInput")
    tm_d = nc.dram_tensor("trimask", [128, 128], BF16, kind="ExternalInput")
    # bpack: bq | bk | bvb  (f32)
    bp_d = nc.dram_tensor("bpack", [128, 130], F32, kind="ExternalInput")
    xt_d = nc.dram_tensor("xt", [128, NQ, KD, 512], BF16, kind="ExternalInput")
    at_d = nc.dram_tensor("at_out", [128, T], BF16, kind="ExternalOutput")

    with tile.TileContext(nc) as tc, ExitStack() as ctx:
        const = ctx.enter_context(tc.tile_pool(name="const", bufs=1))
        persist = ctx.enter_context(tc.tile_pool(name="persist", bufs=1))
        xtp = ctx.enter_context(tc.tile_pool(name="xtp", bufs=2))
        e_pool = ctx.enter_context(tc.tile_pool(name="e_pool", bufs=4))
        rb_pool = ctx.enter_context(tc.tile_pool(name="rb_pool", bufs=2))
        qkv_ps = ctx.enter_context(tc.tile_pool(name="qkv_ps", bufs=2, space="PSUM"))
        s_ps = ctx.enter_context(tc.tile_pool(name="s_ps", bufs=2, space="PSUM"))
        pv_ps = ctx.enter_context(tc.tile_pool(name="pv_ps", bufs=1, space="PSUM"))

        wp_sb = const.tile([128, 3, KD, 128], BF16, tag="wp")
        nc.sync.dma_start(wp_sb[:, 0], wp_d.ap()[:, 0])
        wq_sb, wk_sb, wv_sb = wp_sb[:, 0], wp_sb[:, 1], wp_sb[:, 2]

        # first data chunk, per-kt slices so the very first matmul can start
        # after ~128KB instead of after the whole chunk
        xt_tiles = []
        xt_c0 = xtp.tile([128, KD, 512], BF16, tag="xt", name="xt_0")
        for kt in range(KD):
            nc.sync.dma_start(xt_c0[:, kt], xt_d.ap()[:, 0, kt])
        xt_tiles.append(xt_c0)
        nc.sync.dma_start(wp_sb[:, 1:3], wp_d.ap()[:, 1:3])

        bp_sb = const.tile([128, 130], F32, tag="bp")
        nc.sync.dma_start(bp_sb[:], bp_d.ap())
        bq_sb, bk_sb, bvb_sb = bp_sb[:, 0:1], bp_sb[:, 1:2], bp_sb[:, 2:130]
        trimask = const.tile([128, 128], BF16, tag="tm")
        nc.sync.dma_start(trimask[:], tm_d.ap())

        qt_sb = persist.tile([128, T], BF16, tag="qt")
        kt_sb = persist.tile([128, T], BF16, tag="kt")
        # V natural layout per 128-token tile: 64 V cols + ones + zero, per head
        v_sb = persist.tile([128, T // 128, 132], BF16, tag="v")
        nc.gpsimd.memset(v_sb[:, :, 64:65], 1.0)
        nc.gpsimd.memset(v_sb[:, :, 65:66], 0.0)
        nc.gpsimd.memset(v_sb[:, :, 130:131], 1.0)
        nc.gpsimd.memset(v_sb[:, :, 131:132], 0.0)
        at_sb = persist.tile([128, T], BF16, tag="at")

        def do_qkv(c):
            c_sl = slice(c * 512, (c + 1) * 512)
            if c < len(xt_tiles):
                xt_c = xt_tiles[c]
            else:
                xt_c = xtp.tile([128, KD, 512], BF16, tag="xt", name=f"xt_{c}")
                nc.sync.dma_start(xt_c[:], xt_d.ap()[:, c])

            for w_sb, b_sb, dst in ((wq_sb, bq_sb, qt_sb), (wk_sb, bk_sb, kt_sb)):
                pp = qkv_ps.tile([128, 512], F32, tag="pp", name=f"pp_{c}_{dst.name}")
                for kt in range(KD):
                    nc.tensor.matmul(pp[:], w_sb[:, kt, :], xt_c[:, kt, :],
                                     start=(kt == 0), stop=(kt == KD - 1))
                nc.vector.tensor_scalar(out=dst[:, c_sl], in0=pp[:],
                                        scalar1=b_sb, scalar2=None, op0=ADD)
            for tt in range(4):
                t_tile = c * 4 + tt
                vp = qkv_ps.tile([128, 128], F32, tag="pp", name=f"vp_{t_tile}")
                for kt in range(KD):
                    nc.tensor.matmul(vp[:], xt_c[:, kt, tt * 128:(tt + 1) * 128],
                                     wv_sb[:, kt, :],
                                     start=(kt == 0), stop=(kt == KD - 1))
                dst = v_sb[:, t_tile, :].rearrange("p (a b) -> p a b", a=2)[:, :, 0:64]
                nc.vector.tensor_tensor(
                    out=dst, in0=vp[:].rearrange("p (a b) -> p a b", a=2),
                    in1=bvb_sb.rearrange("p (a b) -> p a b", a=2),
                    op=ADD)
            # prefetch next chunk (issued late so it doesn't steal DMA
            # bandwidth from the chunk currently being consumed)
            if c + 1 == len(xt_tiles) and c + 1 < NQ:
                xt_n = xtp.tile([128, KD, 512], BF16, tag="xt", name=f"xt_{c + 1}")
                nc.sync.dma_start(xt_n[:], xt_d.ap()[:, c + 1])
                xt_tiles.append(xt_n)

        def do_attention(c):
            c_sl = slice(c * 512, (c + 1) * 512)
            nkt = 4 * (c + 1)
            pv = [pv_ps.tile([66, 512], F32, tag=f"pv{h}", name=f"pv{h}_{c}")
                  for h in (0, 1)]

            def emit_pv(kt, esb, o):
                for h in (0, 1):
                    nc.tensor.matmul(pv[h][:, o:512],
                                     v_sb[:, kt, 66 * h:66 * h + 66],
                                     esb[:, h, o:512],
                                     start=(kt == 0), stop=(kt == nkt - 1),
                                     skip_group_check=True)

            prev = None
            prev_o = 0
            for kt in range(nkt):
                o = max(0, kt * 128 - c * 512)
                diag = kt >= nkt - 4
                sp = s_ps.tile([128, 2, 512], F32, tag="s", name=f"s_{c}_{kt}")
                for h in (0, 1):
                    h_sl = slice(64 * h, 64 * h + 64)
                    nc.tensor.matmul(sp[:, h, o:512],
                                     kt_sb[h_sl, kt * 128:(kt + 1) * 128],
                                     qt_sb[h_sl, c * 512 + o:(c + 1) * 512],
                                     start=True, stop=True)
                esb = e_pool.tile([128, 2, 512], BF16, tag="e", name=f"e_{c}_{kt}")
                if not diag and (kt % 2 == 1):
                    # Schraudolph exp on DVE: bf16 bits via int16 output
                    nc.vector.tensor_scalar(out=esb[:].bitcast(I16), in0=sp[:],
                                            scalar1=A16, scalar2=B16,
                                            op0=MULT, op1=ADD)
                elif not diag:
                    nc.scalar.activation(out=esb[:], in_=sp[:], func=EXP)
                else:
                    if o == 0:
                        nc.scalar.activation(out=esb[:], in_=sp[:], func=EXP)
                    else:
                        for h in (0, 1):
                            nc.scalar.activation(out=esb[:, h, o:512],
                                                 in_=sp[:, h, o:512], func=EXP)
                    for h in (0, 1):
                        nc.vector.tensor_mul(esb[:, h, o:o + 128],
                                             esb[:, h, o:o + 128], trimask[:])
                if prev is not None:
                    emit_pv(kt - 1, prev, prev_o)
                prev, prev_o = esb, o
            emit_pv(nkt - 1, prev, prev_o)

            # epilogue: denominator broadcast + reciprocal + normalize
            for h in (0, 1):
                r1 = rb_pool.tile([1, 512], F32, tag="r1", name=f"r1{h}_{c}")
                nc.vector.tensor_copy(r1[:], pv[h][64:65, :])
                rb = rb_pool.tile([128, 512], F32, tag="rb", name=f"rb{h}_{c}")
                nc.gpsimd.partition_broadcast(rb[:], r1[:], channels=128)
                nc.vector.reciprocal_approx_fast(out=rb[:], in_=rb[:])
                nc.vector.tensor_mul(at_sb[64 * h:64 * h + 64, c_sl],
                                     pv[h][0:64, :], rb[64 * h:64 * h + 64, :])
            nc.sync.dma_start(at_d.ap()[:, c_sl], at_sb[:, c_sl])

        # software pipeline: QKV runs one chunk ahead of attention so scores
        # never wait on freshly written Q/K/V.
        for c in range(NQ):
            do_qkv(c)
            if c >= 1:
                do_attention(c - 1)
        do_attention(NQ - 1)

    nc.compile()
    return nc


def build_kernel_b(T=T_FULL, ln_affine=False):
    """Per core: slice of T/8 tokens: out-proj + residual(+bout folded on host
    into xb) + LayerNorm (gamma/beta applied only if ln_affine)."""
    nc = bacc.Bacc("TRN2", target_bir_lowering=False, debug=False)
    Tc = T // NCORES
    KD = D // 128
    IDENT = mybir.ActivationFunctionType.Identity

    at_d = nc.dram_tensor("at", [128, KD, Tc], BF16, kind="ExternalInput")
    wo_d = nc.dram_tensor("wout", [128, 2, KD, 512], BF16, kind="ExternalInput")
    xb_d = nc.dram_tensor("xb", [Tc, D], BF16, kind="ExternalInput")
    g_d = nc.dram_tensor("gamma", [128, D], F32, kind="ExternalInput")
    be_d = nc.dram_tensor("beta", [128, D], F32, kind="ExternalInput")
    y_d = nc.dram_tensor("y", [Tc, D], F32, kind="ExternalOutput")

    with tile.TileContext(nc) as tc, ExitStack() as ctx:
        const = ctx.enter_context(tc.tile_pool(name="const", bufs=1))
        work = ctx.enter_context(tc.tile_pool(name="work", bufs=2))
        stats = ctx.enter_context(tc.tile_pool(name="stats", bufs=4))
        ps = ctx.enter_context(tc.tile_pool(name="ps", bufs=4, space="PSUM"))

        # interleave at / wout-half0 DMAs so the first matmuls start early
        at_sb = const.tile([128, KD, Tc], BF16, tag="at")
        wo_sb = const.tile([128, 2, KD, 512], BF16, tag="wo")
        nc.sync.dma_start(at_sb[:, 0:4], at_d.ap()[:, 0:4])
        nc.sync.dma_start(wo_sb[:, 0, 0:4], wo_d.ap()[:, 0, 0:4])
        nc.sync.dma_start(at_sb[:, 4:8], at_d.ap()[:, 4:8])
        nc.sync.dma_start(wo_sb[:, 0, 4:8], wo_d.ap()[:, 0, 4:8])
        nc.sync.dma_start(wo_sb[:, 1], wo_d.ap()[:, 1])
        xb_tiles = []
        for tt in range(Tc // 128):
            xb_t = work.tile([128, D], BF16, tag="xb", name=f"xb_{tt}")
            nc.sync.dma_start(xb_t[:], xb_d.ap()[tt * 128:(tt + 1) * 128, :])
            xb_tiles.append(xb_t)
        if ln_affine:
            gam_b = const.tile([128, D], F32, tag="gam")
            bet_b = const.tile([128, D], F32, tag="bet")
            nc.sync.dma_start(gam_b[:], g_d.ap())
            nc.sync.dma_start(bet_b[:], be_d.ap())
        eps_sb = const.tile([128, 1], F32, tag="eps")
        nc.vector.memset(eps_sb[:], LN_EPS)

        for tt in range(Tc // 128):
            t_sl = slice(tt * 128, (tt + 1) * 128)
            xb_t = xb_tiles[tt]
            y_t = work.tile([128, D], F32, tag="y")
            for j in (0, 1):
                pp = ps.tile([128, 512], F32, tag="pp")
                for kt in range(KD):
                    nc.tensor.matmul(pp[:], at_sb[:, kt, t_sl],
                                     wo_sb[:, j, kt, :],
                                     start=(kt == 0), stop=(kt == KD - 1))
                nc.vector.tensor_add(y_t[:, j * 512:(j + 1) * 512], pp[:],
                                     xb_t[:, j * 512:(j + 1) * 512])
            st = stats.tile([128, 2, 6], F32, tag="st")
            nc.vector.bn_stats(st[:, 0, :], y_t[:, 0:512])
            nc.vector.bn_stats(st[:, 1, :], y_t[:, 512:1024])
            mv = stats.tile([128, 2], F32, tag="mv")
            nc.vector.bn_aggr(mv[:], st[:])
            sq = stats.tile([128, 1], F32, tag="sq")
            nc.scalar.activation(out=sq[:], in_=mv[:, 1:2], func=SQRT,
                                 bias=eps_sb[:], scale=1.0)
            rstd = stats.tile([128, 1], F32, tag="rstd")
            nc.vector.reciprocal(rstd[:], sq[:])
            # nm = -mu * rstd;  y = y * rstd + nm   (one ACT op)
            nm = stats.tile([128, 1], F32, tag="nm")
            nc.vector.tensor_scalar(out=nm[:], in0=mv[:, 0:1],
                                    scalar1=rstd[:], scalar2=-1.0,
                                    op0=MULT, op1=MULT)
            nc.scalar.activation(out=y_t[:], in_=y_t[:], func=IDENT,
                                 bias=nm[:], scale=rstd[:])
            if ln_affine:
                nc.vector.tensor_mul(y_t[:], y_t[:], gam_b[:])
                nc.vector.tensor_add(y_t[:], y_t[:], bet_b[:])
            nc.sync.dma_start(y_d.ap()[t_sl, :], y_t[:])

    nc.compile()
    return nc


def _get_kernels(T=T_FULL, ln_affine=False):
    key = (T, ln_affine)
    if key not in _CACHE:
        _CACHE[key] = (build_kernel_a(T), build_kernel_b(T, ln_affine))
    return _CACHE[key]


def kernel(x, Wqkv, bqkv, Wout, bout, gamma, beta):
    x = np.asarray(x, dtype=np.float32)
    Wqkv = np.asarray(Wqkv, dtype=np.float32)
    bqkv = np.asarray(bqkv, dtype=np.float32)
    Wout = np.asarray(Wout, dtype=np.float32)
    bout = np.asarray(bout, dtype=np.float32)
    gamma = np.asarray(gamma, dtype=np.float32)
    beta = np.asarray(beta, dtype=np.float32)

    B, T, D_ = x.shape
    assert B == 1 and D_ == D
    d = D // HEADS
    scale = d ** -0.5
    x2d = np.ascontiguousarray(x[0])
    KD = D // 128
    NQ = T // 512

    # host-side layout prep (free): xt[p, c, k, j] = x[c*512+j, k*128+p]
    xt = np.ascontiguousarray(
        x2d.T.reshape(KD, 128, NQ, 512).transpose(1, 2, 0, 3)).astype(BF16NP)
    trimask = np.triu(np.ones((128, 128), np.float32)).astype(BF16NP)

    ln_affine = not (np.all(gamma == 1.0) and np.all(beta == 0.0))
    nc_a, nc_b = _get_kernels(T, ln_affine)

    in_maps_a = []
    for c in range(NCORES):
        r = slice(c * 128, (c + 1) * 128)
        wq = Wqkv[0 * D:1 * D][r]            # [128, D]
        wk = Wqkv[1 * D:2 * D][r] * scale
        wv = Wqkv[2 * D:3 * D][r]
        bv = bqkv[2 * D:3 * D][r]
        # stationary layout [128 part=D-slice, kt, 128 out]
        wpack = np.stack([
            w.T.reshape(KD, 128, 128).transpose(1, 0, 2)
            for w in (wq, wk, wv)], axis=1)  # [128, 3, KD, 128]
        bpack = np.concatenate([
            bqkv[0 * D:1 * D][r].reshape(128, 1),
            (bqkv[1 * D:2 * D][r] * scale).reshape(128, 1),
            np.tile(bv.reshape(1, 128), (128, 1)),
        ], axis=1)  # [128, 130]
        in_maps_a.append({
            "xt": xt,
            "trimask": trimask,
            "wpack": np.ascontiguousarray(wpack).astype(BF16NP),
            "bpack": np.ascontiguousarray(bpack),
        })
    res_a = run_bass_kernel_spmd(nc_a, in_maps_a, core_ids=list(range(NCORES)))
    LAST_RESULTS["a"] = res_a
    at_full = np.concatenate([res_a.results[c]["at_out"] for c in range(NCORES)],
                             axis=0)  # [D, T] bf16

    Tc = T // NCORES
    wout_st = np.ascontiguousarray(
        Wout.T.reshape(KD, 128, 2, 512).transpose(1, 2, 0, 3)).astype(BF16NP)
    gam_rep = np.ascontiguousarray(np.tile(gamma.reshape(1, D), (128, 1)))
    bet_rep = np.ascontiguousarray(np.tile(beta.reshape(1, D), (128, 1)))
    in_maps_b = []
    for c in range(NCORES):
        t_sl = slice(c * Tc, (c + 1) * Tc)
        at_c = at_full[:, t_sl]  # [D, Tc] bf16
        in_maps_b.append({
            "at": np.ascontiguousarray(at_c.reshape(KD, 128, Tc).transpose(1, 0, 2)),
            "wout": wout_st,
            "xb": np.ascontiguousarray(x2d[t_sl] + bout[None, :]).astype(BF16NP),
            "gamma": gam_rep,
            "beta": bet_rep,
        })
    res_b = run_bass_kernel_spmd(nc_b, in_maps_b, core_ids=list(range(NCORES)))
    LAST_RESULTS["b"] = res_b
    y = np.concatenate([res_b.results[c]["y"] for c in range(NCORES)], axis=0)
    return y.reshape(1, T, D).astype(np.float32)


# revision 31
# speedup vs baseline: 1.1502x; 1.1238x over previous
"""Trainium2 Bass kernel for causal self-attention + out-proj + residual + LayerNorm.

v3: heads (tensor-parallel) across 8 cores for QKV+attention (kernel A),
then sequence-parallel across 8 cores for out-proj + residual + LN (kernel B).

Design:
- x is transposed + cast to bf16 on the HOST (free): no on-device transposes.
  Chunk-contiguous layout so each DMA is 8 KB/partition contiguous.
- All matmul operands bf16 (FWL weight loads, half SBUF/DMA traffic).
- QKV projection is chunk-pipelined with attention (hides under the exp wall).
- Causal trimming: diagonal-chunk score/PV matmuls and exps only touch the
  valid column range.
- exp split across Scalar (ACT true exp) and Vector (DVE Schraudolph bit-trick
  exp -> bf16 bits via int16 output): the two engines run concurrently.
- V computed directly in [token, dim] layout (stationary = xT tile), softmax
  denominator via an appended ones column.
- pv PSUM released early via ACT-engine copy to SBUF; normalize mul and
  diagonal mask mul run on the otherwise-idle GPSIMD engine.
"""

import math
from contextlib import ExitStack

import numpy as np
import ml_dtypes

import concourse.bass as bass
import concourse.tile as tile
from concourse import bacc, mybir
from concourse.bass_utils import run_bass_kernel_spmd

BF16NP = ml_dtypes.bfloat16


# NTFF-trace shim: make run_bass_kernel_spmd(trace=True) usable in containers
# whose antenv lacks axon_hooks (harmless when tracing is off).
def _install_trace_shim():
    import sys, types
    try:
        import antenv.axon_hooks  # noqa: F401
        return
    except ImportError:
        pass
    try:
        import antenv
        from trn_agent_boot.trn_boot import _ntff_profile_via_ctypes
        hook = _ntff_profile_via_ctypes("/opt/axon/libaxon_pjrt.so")
        mod = types.ModuleType("antenv.axon_hooks")
        mod.get_axon_ntff_profile_hook = lambda: hook
        mod.set_axon_ntff_profile_hook = lambda h: None
        sys.modules["antenv.axon_hooks"] = mod
        antenv.axon_hooks = mod
        import concourse.bass_utils as _bu
        _bu.upload_artifacts = lambda tmpdir: "local://skipped"
    except Exception:
        pass


_install_trace_shim()

F32 = mybir.dt.float32
BF16 = mybir.dt.bfloat16
I16 = mybir.dt.int16
EXP = mybir.ActivationFunctionType.Exp
COPYF = mybir.ActivationFunctionType.Copy
SQRT = mybir.ActivationFunctionType.Sqrt
IDENT = mybir.ActivationFunctionType.Identity
ADD = mybir.AluOpType.add
MULT = mybir.AluOpType.mult
SUB = mybir.AluOpType.subtract

T_FULL = 4096
D = 1024
HEADS = 16
NCORES = 8
LN_EPS = 1e-5

# Schraudolph exp -> bf16 bit pattern via int16: exp(x) ~= bf16_bits(int16(x*A16 + B16))
A16 = 128.0 / math.log(2.0)
B16 = 16251.0  # tuned for truncation toward zero on positive values

_CACHE = {}
LAST_RESULTS = {}


def build_kernel_a(T=T_FULL):
    """Per core: 2 heads. Computes at = softmax(QK^T/sqrt(d)) @ V in layout
    [128 = 2*64 head dims, T], bf16, normalized."""
    nc = bacc.Bacc("TRN2", target_bir_lowering=False, debug=False)
    KD = D // 128          # 8 contraction tiles over D
    NQ = T // 512          # chunks of 512 tokens

    # wpack: wq|wk|wv stationary tiles + trimask, single bf16 DMA
    wp_d = nc.dram_tensor("wpack", [128, 3, KD, 128], BF16, kind="ExternalInput")
    tm_d = nc.dram_tensor("trimask", [128, 128], BF16, kind="ExternalInput")
    # bpack: bq | bk | bvb  (f32)
    bp_d = nc.dram_tensor("bpack", [128, 130], F32, kind="ExternalInput")
    xt_d = nc.dram_tensor("xt", [128, NQ, KD, 512], BF16, kind="ExternalInput")
    at_d = nc.dram_tensor("at_out", [128, T], BF16, kind="ExternalOutput")

    with tile.TileContext(nc) as tc, ExitStack() as ctx:
        const = ctx.enter_context(tc.tile_pool(name="const", bufs=1))
        persist = ctx.enter_context(tc.tile_pool(name="persist", bufs=1))
        xtp = ctx.enter_context(tc.tile_pool(name="xtp", bufs=2))
        e_pool = ctx.enter_context(tc.tile_pool(name="e_pool", bufs=4))
        rb_pool = ctx.enter_context(tc.tile_pool(name="rb_pool", bufs=2))
        qkv_ps = ctx.enter_context(tc.tile_pool(name="qkv_ps", bufs=2, space="PSUM"))
        s_ps = ctx.enter_context(tc.tile_pool(name="s_ps", bufs=2, space="PSUM"))
        pv_ps = ctx.enter_context(tc.tile_pool(name="pv_ps", bufs=1, space="PSUM"))

        wp_sb = const.tile([128, 3, KD, 128], BF16, tag="wp")
        nc.sync.dma_start(wp_sb[:, 0], wp_d.ap()[:, 0])
        wq_sb, wk_sb, wv_sb = wp_sb[:, 0], wp_sb[:, 1], wp_sb[:, 2]

        # first data chunk, per-kt slices so the very first matmul can start
        # after ~128KB instead of after the whole chunk
        xt_tiles = []
        xt_c0 = xtp.tile([128, KD, 512], BF16, tag="xt", name="xt_0")
        for kt in range(KD):
            nc.sync.dma_start(xt_c0[:, kt], xt_d.ap()[:, 0, kt])
        xt_tiles.append(xt_c0)
        nc.sync.dma_start(wp_sb[:, 1:3], wp_d.ap()[:, 1:3])

        bp_sb = const.tile([128, 130], F32, tag="bp")
        nc.sync.dma_start(bp_sb[:], bp_d.ap())
        bq_sb, bk_sb, bvb_sb = bp_sb[:, 0:1], bp_sb[:, 1:2], bp_sb[:, 2:130]
        trimask = const.tile([128, 128], BF16, tag="tm")
        nc.sync.dma_start(trimask[:], tm_d.ap())

        qt_sb = persist.tile([128, T], BF16, tag="qt")
        kt_sb = persist.tile([128, T], BF16, tag="kt")
        # V natural layout per 128-token tile: 64 V cols + ones + zero, per head
        v_sb = persist.tile([128, T // 128, 132], BF16, tag="v")
        nc.gpsimd.memset(v_sb[:, :, 64:65], 1.0)
        nc.gpsimd.memset(v_sb[:, :, 65:66], 0.0)
        nc.gpsimd.memset(v_sb[:, :, 130:131], 1.0)
        nc.gpsimd.memset(v_sb[:, :, 131:132], 0.0)
        at_sb = persist.tile([128, T], BF16, tag="at")

        def qkv_emitters(c):
            """Return closures, each emitting one PE work-group of chunk c's
            QKV; they get interleaved between attention k-steps."""
            c_sl = slice(c * 512, (c + 1) * 512)
            if c < len(xt_tiles):
                xt_c = xt_tiles[c]
            else:
                xt_c = xtp.tile([128, KD, 512], BF16, tag="xt", name=f"xt_{c}")
                nc.sync.dma_start(xt_c[:], xt_d.ap()[:, c])
                xt_tiles.append(xt_c)

            def proj(w_sb, b_sb, dst):
                def run():
                    pp = qkv_ps.tile([128, 512], F32, tag="pp",
                                     name=f"pp_{c}_{dst.name}")
                    for kt in range(KD):
                        nc.tensor.matmul(pp[:], w_sb[:, kt, :], xt_c[:, kt, :],
                                         start=(kt == 0), stop=(kt == KD - 1))
                    # bias + cast on ACT (per-partition bias AP)
                    nc.scalar.activation(out=dst[:, c_sl], in_=pp[:],
                                         func=IDENT, bias=b_sb, scale=1.0)
                return run

            def vproj(tt):
                def run():
                    t_tile = c * 4 + tt
                    vp = qkv_ps.tile([128, 128], F32, tag="pp",
                                     name=f"vp_{t_tile}")
                    for kt in range(KD):
                        nc.tensor.matmul(vp[:],
                                         xt_c[:, kt, tt * 128:(tt + 1) * 128],
                                         wv_sb[:, kt, :],
                                         start=(kt == 0), stop=(kt == KD - 1))
                    dst = v_sb[:, t_tile, :].rearrange(
                        "p (a b) -> p a b", a=2)[:, :, 0:64]
                    nc.vector.tensor_tensor(
                        out=dst, in0=vp[:].rearrange("p (a b) -> p a b", a=2),
                        in1=bvb_sb.rearrange("p (a b) -> p a b", a=2),
                        op=ADD)
                return run

            ems = [proj(wq_sb, bq_sb, qt_sb), proj(wk_sb, bk_sb, kt_sb)]
            ems += [vproj(tt) for tt in range(4)]

            def prefetch():
                if c + 1 == len(xt_tiles) and c + 1 < NQ:
                    xt_n = xtp.tile([128, KD, 512], BF16, tag="xt",
                                    name=f"xt_{c + 1}")
                    nc.sync.dma_start(xt_n[:], xt_d.ap()[:, c + 1])
                    xt_tiles.append(xt_n)
            ems.append(prefetch)
            return ems

        def do_attention(c, ems):
            c_sl = slice(c * 512, (c + 1) * 512)
            nkt = 4 * (c + 1)
            pv = [pv_ps.tile([66, 512], F32, tag=f"pv{h}", name=f"pv{h}_{c}")
                  for h in (0, 1)]

            def emit_pv(kt, es, o):
                for h in (0, 1):
                    nc.tensor.matmul(pv[h][:, o:512],
                                     v_sb[:, kt, 66 * h:66 * h + 66],
                                     es[h][:, o:512],
                                     start=(kt == 0), stop=(kt == nkt - 1),
                                     skip_group_check=True)

            prev = None
            prev_o = 0
            for kt in range(nkt):
                if ems:
                    ems.pop(0)()
                o = max(0, kt * 128 - c * 512)
                diag = kt >= nkt - 4
                es = []
                for h in (0, 1):
                    h_sl = slice(64 * h, 64 * h + 64)
                    sp = s_ps.tile([128, 512], F32, tag=f"s{h}",
                                   name=f"s{h}_{c}_{kt}")
                    nc.tensor.matmul(sp[:, o:512],
                                     kt_sb[h_sl, kt * 128:(kt + 1) * 128],
                                     qt_sb[h_sl, c * 512 + o:(c + 1) * 512],
                                     start=True, stop=True)
                    esb = e_pool.tile([128, 512], BF16, tag=f"e{h}",
                                      name=f"e{h}_{c}_{kt}")
                    if h == 0:
                        nc.scalar.activation(out=esb[:, o:512],
                                             in_=sp[:, o:512], func=EXP)
                    else:
                        # Schraudolph exp on DVE: bf16 bits via int16 output
                        nc.vector.tensor_scalar(
                            out=esb[:, o:512].bitcast(I16), in0=sp[:, o:512],
                            scalar1=A16, scalar2=B16, op0=MULT, op1=ADD)
                    if diag:
                        nc.vector.tensor_mul(esb[:, o:o + 128],
                                             esb[:, o:o + 128], trimask[:])
                    es.append(esb)
                if prev is not None:
                    emit_pv(kt - 1, prev, prev_o)
                prev, prev_o = es, o
            emit_pv(nkt - 1, prev, prev_o)
            while ems:
                ems.pop(0)()

            # epilogue: denominator broadcast + reciprocal + normalize
            for h in (0, 1):
                r1 = rb_pool.tile([1, 512], F32, tag="r1", name=f"r1{h}_{c}")
                nc.vector.tensor_copy(r1[:], pv[h][64:65, :])
                rb = rb_pool.tile([128, 512], F32, tag="rb", name=f"rb{h}_{c}")
                nc.gpsimd.partition_broadcast(rb[:], r1[:], channels=128)
                nc.vector.reciprocal_approx_fast(out=rb[:], in_=rb[:])
                nc.vector.tensor_mul(at_sb[64 * h:64 * h + 64, c_sl],
                                     pv[h][0:64, :], rb[64 * h:64 * h + 64, :])
            nc.sync.dma_start(at_d.ap()[:, c_sl], at_sb[:, c_sl])

        # software pipeline: chunk c's QKV work-groups are interleaved into
        # the k-steps of attention on chunk c-1.
        for em in qkv_emitters(0):
            em()
        for c in range(1, NQ):
            do_attention(c - 1, qkv_emitters(c))
        do_attention(NQ - 1, [])

    nc.compile()
    return nc


def build_kernel_b(T=T_FULL, ln_affine=False):
    """Per core: slice of T/8 tokens: out-proj + residual(+bout folded on host
    into xb) + LayerNorm (gamma/beta applied only if ln_affine)."""
    nc = bacc.Bacc("TRN2", target_bir_lowering=False, debug=False)
    Tc = T // NCORES
    KD = D // 128
    IDENT = mybir.ActivationFunctionType.Identity

    at_d = nc.dram_tensor("at", [128, KD, Tc], BF16, kind="ExternalInput")
    wo_d = nc.dram_tensor("wout", [128, 2, KD, 512], BF16, kind="ExternalInput")
    xb_d = nc.dram_tensor("xb", [Tc, D], BF16, kind="ExternalInput")
    g_d = nc.dram_tensor("gamma", [128, D], F32, kind="ExternalInput")
    be_d = nc.dram_tensor("beta", [128, D], F32, kind="ExternalInput")
    y_d = nc.dram_tensor("y", [Tc, D], F32, kind="ExternalOutput")

    with tile.TileContext(nc) as tc, ExitStack() as ctx:
        const = ctx.enter_context(tc.tile_pool(name="const", bufs=1))
        work = ctx.enter_context(tc.tile_pool(name="work", bufs=2))
        stats = ctx.enter_context(tc.tile_pool(name="stats", bufs=4))
        ps = ctx.enter_context(tc.tile_pool(name="ps", bufs=4, space="PSUM"))

        # interleave at / wout-half0 DMAs so the first matmuls start early
        at_sb = const.tile([128, KD, Tc], BF16, tag="at")
        wo_sb = const.tile([128, 2, KD, 512], BF16, tag="wo")
        nc.sync.dma_start(at_sb[:, 0:4], at_d.ap()[:, 0:4])
        nc.sync.dma_start(wo_sb[:, 0, 0:4], wo_d.ap()[:, 0, 0:4])
        nc.sync.dma_start(at_sb[:, 4:8], at_d.ap()[:, 4:8])
        nc.sync.dma_start(wo_sb[:, 0, 4:8], wo_d.ap()[:, 0, 4:8])
        nc.sync.dma_start(wo_sb[:, 1], wo_d.ap()[:, 1])
        xb_tiles = []
        for tt in range(Tc // 128):
            xb_t = work.tile([128, D], BF16, tag="xb", name=f"xb_{tt}")
            nc.sync.dma_start(xb_t[:], xb_d.ap()[tt * 128:(tt + 1) * 128, :])
            xb_tiles.append(xb_t)
        if ln_affine:
            gam_b = const.tile([128, D], F32, tag="gam")
            bet_b = const.tile([128, D], F32, tag="bet")
            nc.sync.dma_start(gam_b[:], g_d.ap())
            nc.sync.dma_start(bet_b[:], be_d.ap())
        eps_sb = const.tile([128, 1], F32, tag="eps")
        nc.vector.memset(eps_sb[:], LN_EPS)

        for tt in range(Tc // 128):
            t_sl = slice(tt * 128, (tt + 1) * 128)
            xb_t = xb_tiles[tt]
            y_t = work.tile([128, D], F32, tag="y")
            for j in (0, 1):
                pp = ps.tile([128, 512], F32, tag="pp")
                for kt in range(KD):
                    nc.tensor.matmul(pp[:], at_sb[:, kt, t_sl],
                                     wo_sb[:, j, kt, :],
                                     start=(kt == 0), stop=(kt == KD - 1))
                nc.vector.tensor_add(y_t[:, j * 512:(j + 1) * 512], pp[:],
                                     xb_t[:, j * 512:(j + 1) * 512])
            st = stats.tile([128, 2, 6], F32, tag="st")
            nc.vector.bn_stats(st[:, 0, :], y_t[:, 0:512])
            nc.vector.bn_stats(st[:, 1, :], y_t[:, 512:1024])
            mv = stats.tile([128, 2], F32, tag="mv")
            nc.vector.bn_aggr(mv[:], st[:])
            sq = stats.tile([128, 1], F32, tag="sq")
            nc.scalar.activation(out=sq[:], in_=mv[:, 1:2], func=SQRT,
                                 bias=eps_sb[:], scale=1.0)
            rstd = stats.tile([128, 1], F32, tag="rstd")
            nc.vector.reciprocal(rstd[:], sq[:])
            # nm = -mu * rstd;  y = y * rstd + nm   (one ACT op)
            nm = stats.tile([128, 1], F32, tag="nm")
            nc.vector.tensor_scalar(out=nm[:], in0=mv[:, 0:1],
                                    scalar1=rstd[:], scalar2=-1.0,
                                    op0=MULT, op1=MULT)
            nc.scalar.activation(out=y_t[:], in_=y_t[:], func=IDENT,
                                 bias=nm[:], scale=rstd[:])
            if ln_affine:
                nc.vector.tensor_mul(y_t[:], y_t[:], gam_b[:])
                nc.vector.tensor_add(y_t[:], y_t[:], bet_b[:])
            nc.sync.dma_start(y_d.ap()[t_sl, :], y_t[:])

    nc.compile()
    return nc


def _get_kernels(T=T_FULL, ln_affine=False):
    key = (T, ln_affine)
    if key not in _CACHE:
        _CACHE[key] = (build_kernel_a(T), build_kernel_b(T, ln_affine))
    return _CACHE[key]


def kernel(x, Wqkv, bqkv, Wout, bout, gamma, beta):
    x = np.asarray(x, dtype=np.float32)
    Wqkv = np.asarray(Wqkv, dtype=np.float32)
    bqkv = np.asarray(bqkv, dtype=np.float32)
    Wout = np.asarray(Wout, dtype=np.float32)
    bout = np.asarray(bout, dtype=np.float32)
    gamma = np.asarray(gamma, dtype=np.float32)
    beta = np.asarray(beta, dtype=np.float32)

    B, T, D_ = x.shape
    assert B == 1 and D_ == D
    d = D // HEADS
    scale = d ** -0.5
    x2d = np.ascontiguousarray(x[0])
    KD = D // 128
    NQ = T // 512

    # host-side layout prep (free): xt[p, c, k, j] = x[c*512+j, k*128+p]
    xt = np.ascontiguousarray(
        x2d.T.reshape(KD, 128, NQ, 512).transpose(1, 2, 0, 3)).astype(BF16NP)
    trimask = np.triu(np.ones((128, 128), np.float32)).astype(BF16NP)

    ln_affine = not (np.all(gamma == 1.0) and np.all(beta == 0.0))
    nc_a, nc_b = _get_kernels(T, ln_affine)

    in_maps_a = []
    for c in range(NCORES):
        r = slice(c * 128, (c + 1) * 128)
        wq = Wqkv[0 * D:1 * D][r]            # [128, D]
        wk = Wqkv[1 * D:2 * D][r] * scale
        wv = Wqkv[2 * D:3 * D][r]
        bv = bqkv[2 * D:3 * D][r]
        # stationary layout [128 part=D-slice, kt, 128 out]
        wpack = np.stack([
            w.T.reshape(KD, 128, 128).transpose(1, 0, 2)
            for w in (wq, wk, wv)], axis=1)  # [128, 3, KD, 128]
        bpack = np.concatenate([
            bqkv[0 * D:1 * D][r].reshape(128, 1),
            (bqkv[1 * D:2 * D][r] * scale).reshape(128, 1),
            np.tile(bv.reshape(1, 128), (128, 1)),
        ], axis=1)  # [128, 130]
        in_maps_a.append({
            "xt": xt,
            "trimask": trimask,
            "wpack": np.ascontiguousarray(wpack).astype(BF16NP),
            "bpack": np.ascontiguousarray(bpack),
        })
    res_a = run_bass_kernel_spmd(nc_a, in_maps_a, core_ids=list(range(NCORES)))
    LAST_RESULTS["a"] = res_a
    at_full = np.concatenate([res_a.results[c]["at_out"] for c in range(NCORES)],
                             axis=0)  # [D, T] bf16

    Tc = T // NCORES
    wout_st = np.ascontiguousarray(
        Wout.T.reshape(KD, 128, 2, 512).transpose(1, 2, 0, 3)).astype(BF16NP)
    gam_rep = np.ascontiguousarray(np.tile(gamma.reshape(1, D), (128, 1)))
    bet_rep = np.ascontiguousarray(np.tile(beta.reshape(1, D), (128, 1)))
    in_maps_b = []
    for c in range(NCORES):
        t_sl = slice(c * Tc, (c + 1) * Tc)
        at_c = at_full[:, t_sl]  # [D, Tc] bf16
        in_maps_b.append({
            "at": np.ascontiguousarray(at_c.reshape(KD, 128, Tc).transpose(1, 0, 2)),
            "wout": wout_st,
            "xb": np.ascontiguousarray(x2d[t_sl] + bout[None, :]).astype(BF16NP),
            "gamma": gam_rep,
            "beta": bet_rep,
        })
    res_b = run_bass_kernel_spmd(nc_b, in_maps_b, core_ids=list(range(NCORES)))
    LAST_RESULTS["b"] = res_b
    y = np.concatenate([res_b.results[c]["y"] for c in range(NCORES)], axis=0)
    return y.reshape(1, T, D).astype(np.float32)


# revision 32
# speedup vs baseline: 1.1537x; 1.0030x over previous
"""Trainium2 Bass kernel for causal self-attention + out-proj + residual + LayerNorm.

v3: heads (tensor-parallel) across 8 cores for QKV+attention (kernel A),
then sequence-parallel across 8 cores for out-proj + residual + LN (kernel B).

Design:
- x is transposed + cast to bf16 on the HOST (free): no on-device transposes.
  Chunk-contiguous layout so each DMA is 8 KB/partition contiguous.
- All matmul operands bf16 (FWL weight loads, half SBUF/DMA traffic).
- QKV projection is chunk-pipelined with attention (hides under the exp wall).
- Causal trimming: diagonal-chunk score/PV matmuls and exps only touch the
  valid column range.
- exp split across Scalar (ACT true exp) and Vector (DVE Schraudolph bit-trick
  exp -> bf16 bits via int16 output): the two engines run concurrently.
- V computed directly in [token, dim] layout (stationary = xT tile), softmax
  denominator via an appended ones column.
- pv PSUM released early via ACT-engine copy to SBUF; normalize mul and
  diagonal mask mul run on the otherwise-idle GPSIMD engine.
"""

import math
from contextlib import ExitStack

import numpy as np
import ml_dtypes

import concourse.bass as bass
import concourse.tile as tile
from concourse import bacc, mybir
from concourse.bass_utils import run_bass_kernel_spmd

BF16NP = ml_dtypes.bfloat16


# NTFF-trace shim: make run_bass_kernel_spmd(trace=True) usable in containers
# whose antenv lacks axon_hooks (harmless when tracing is off).
def _install_trace_shim():
    import sys, types
    try:
        import antenv.axon_hooks  # noqa: F401
        return
    except ImportError:
        pass
    try:
        import antenv
        from trn_agent_boot.trn_boot import _ntff_profile_via_ctypes
        hook = _ntff_profile_via_ctypes("/opt/axon/libaxon_pjrt.so")
        mod = types.ModuleType("antenv.axon_hooks")
        mod.get_axon_ntff_profile_hook = lambda: hook
        mod.set_axon_ntff_profile_hook = lambda h: None
        sys.modules["antenv.axon_hooks"] = mod
        antenv.axon_hooks = mod
        import concourse.bass_utils as _bu
        _bu.upload_artifacts = lambda tmpdir: "local://skipped"
    except Exception:
        pass


_install_trace_shim()

F32 = mybir.dt.float32
BF16 = mybir.dt.bfloat16
I16 = mybir.dt.int16
EXP = mybir.ActivationFunctionType.Exp
COPYF = mybir.ActivationFunctionType.Copy
SQRT = mybir.ActivationFunctionType.Sqrt
IDENT = mybir.ActivationFunctionType.Identity
ADD = mybir.AluOpType.add
MULT = mybir.AluOpType.mult
SUB = mybir.AluOpType.subtract

T_FULL = 4096
D = 1024
HEADS = 16
NCORES = 8
LN_EPS = 1e-5

# Schraudolph exp -> bf16 bit pattern via int16: exp(x) ~= bf16_bits(int16(x*A16 + B16))
A16 = 128.0 / math.log(2.0)
B16 = 16251.0  # tuned for truncation toward zero on positive values

_CACHE = {}
LAST_RESULTS = {}


def build_kernel_a(T=T_FULL):
    """Per core: 2 heads. Computes at = softmax(QK^T/sqrt(d)) @ V in layout
    [128 = 2*64 head dims, T], bf16, normalized."""
    nc = bacc.Bacc("TRN2", target_bir_lowering=False, debug=False)
    KD = D // 128          # 8 contraction tiles over D
    NQ = T // 512          # chunks of 512 tokens

    # wpack: wq|wk|wv stationary tiles + trimask, single bf16 DMA
    wp_d = nc.dram_tensor("wpack", [128, 3, KD, 128], BF16, kind="ExternalInput")
    tm_d = nc.dram_tensor("trimask", [128, 128], BF16, kind="ExternalInput")
    # bpack: bq | bk | bvb  (f32)
    bp_d = nc.dram_tensor("bpack", [128, 130], F32, kind="ExternalInput")
    xt_d = nc.dram_tensor("xt", [128, NQ, KD, 512], BF16, kind="ExternalInput")
    at_d = nc.dram_tensor("at_out", [128, T], BF16, kind="ExternalOutput")

    with tile.TileContext(nc) as tc, ExitStack() as ctx:
        const = ctx.enter_context(tc.tile_pool(name="const", bufs=1))
        persist = ctx.enter_context(tc.tile_pool(name="persist", bufs=1))
        xtp = ctx.enter_context(tc.tile_pool(name="xtp", bufs=2))
        e_pool = ctx.enter_context(tc.tile_pool(name="e_pool", bufs=4))
        rb_pool = ctx.enter_context(tc.tile_pool(name="rb_pool", bufs=2))
        qkv_ps = ctx.enter_context(tc.tile_pool(name="qkv_ps", bufs=2, space="PSUM"))
        s_ps = ctx.enter_context(tc.tile_pool(name="s_ps", bufs=2, space="PSUM"))
        pv_ps = ctx.enter_context(tc.tile_pool(name="pv_ps", bufs=1, space="PSUM"))

        wp_sb = const.tile([128, 3, KD, 128], BF16, tag="wp")
        nc.sync.dma_start(wp_sb[:, 0], wp_d.ap()[:, 0])
        wq_sb, wk_sb, wv_sb = wp_sb[:, 0], wp_sb[:, 1], wp_sb[:, 2]

        # first data chunk, per-kt slices so the very first matmul can start
        # after ~128KB instead of after the whole chunk
        xt_tiles = []
        xt_c0 = xtp.tile([128, KD, 512], BF16, tag="xt", name="xt_0")
        for kt in range(KD):
            nc.sync.dma_start(xt_c0[:, kt], xt_d.ap()[:, 0, kt])
        xt_tiles.append(xt_c0)
        nc.sync.dma_start(wp_sb[:, 1:3], wp_d.ap()[:, 1:3])

        bp_sb = const.tile([128, 130], F32, tag="bp")
        nc.sync.dma_start(bp_sb[:], bp_d.ap())
        bq_sb, bk_sb, bvb_sb = bp_sb[:, 0:1], bp_sb[:, 1:2], bp_sb[:, 2:130]
        trimask = const.tile([128, 128], BF16, tag="tm")
        nc.sync.dma_start(trimask[:], tm_d.ap())

        qt_sb = persist.tile([128, T], BF16, tag="qt")
        kt_sb = persist.tile([128, T], BF16, tag="kt")
        # V natural layout per 128-token tile: 64 V cols + ones + zero, per head
        v_sb = persist.tile([128, T // 128, 132], BF16, tag="v")
        nc.gpsimd.memset(v_sb[:, :, 64:65], 1.0)
        nc.gpsimd.memset(v_sb[:, :, 65:66], 0.0)
        nc.gpsimd.memset(v_sb[:, :, 130:131], 1.0)
        nc.gpsimd.memset(v_sb[:, :, 131:132], 0.0)
        at_sb = persist.tile([128, T], BF16, tag="at")

        def qkv_emitters(c):
            """Return closures, each emitting one PE work-group of chunk c's
            QKV; they get interleaved between attention k-steps."""
            c_sl = slice(c * 512, (c + 1) * 512)
            if c < len(xt_tiles):
                xt_c = xt_tiles[c]
            else:
                xt_c = xtp.tile([128, KD, 512], BF16, tag="xt", name=f"xt_{c}")
                nc.sync.dma_start(xt_c[:], xt_d.ap()[:, c])
                xt_tiles.append(xt_c)

            def proj(w_sb, b_sb, dst):
                def run():
                    pp = qkv_ps.tile([128, 512], F32, tag="pp",
                                     name=f"pp_{c}_{dst.name}")
                    for kt in range(KD):
                        nc.tensor.matmul(pp[:], w_sb[:, kt, :], xt_c[:, kt, :],
                                         start=(kt == 0), stop=(kt == KD - 1))
                    # bias + cast on ACT (per-partition bias AP)
                    nc.scalar.activation(out=dst[:, c_sl], in_=pp[:],
                                         func=IDENT, bias=b_sb, scale=1.0)
                return run

            def vproj(tt):
                def run():
                    t_tile = c * 4 + tt
                    vp = qkv_ps.tile([128, 128], F32, tag="pp",
                                     name=f"vp_{t_tile}")
                    for kt in range(KD):
                        nc.tensor.matmul(vp[:],
                                         xt_c[:, kt, tt * 128:(tt + 1) * 128],
                                         wv_sb[:, kt, :],
                                         start=(kt == 0), stop=(kt == KD - 1))
                    dst = v_sb[:, t_tile, :].rearrange(
                        "p (a b) -> p a b", a=2)[:, :, 0:64]
                    nc.vector.tensor_tensor(
                        out=dst, in0=vp[:].rearrange("p (a b) -> p a b", a=2),
                        in1=bvb_sb.rearrange("p (a b) -> p a b", a=2),
                        op=ADD)
                return run

            ems = [proj(wq_sb, bq_sb, qt_sb), proj(wk_sb, bk_sb, kt_sb)]
            ems += [vproj(tt) for tt in range(4)]

            def prefetch():
                if c + 1 == len(xt_tiles) and c + 1 < NQ:
                    xt_n = xtp.tile([128, KD, 512], BF16, tag="xt",
                                    name=f"xt_{c + 1}")
                    nc.sync.dma_start(xt_n[:], xt_d.ap()[:, c + 1])
                    xt_tiles.append(xt_n)
            ems.append(prefetch)
            return ems

        def do_attention(c, ems):
            c_sl = slice(c * 512, (c + 1) * 512)
            nkt = 4 * (c + 1)
            pv = [pv_ps.tile([66, 512], F32, tag=f"pv{h}", name=f"pv{h}_{c}")
                  for h in (0, 1)]

            def emit_pv(kt, es, o):
                for h in (0, 1):
                    nc.tensor.matmul(pv[h][:, o:512],
                                     v_sb[:, kt, 66 * h:66 * h + 66],
                                     es[h][:, o:512],
                                     start=(kt == 0), stop=(kt == nkt - 1),
                                     skip_group_check=True)

            prev = None
            prev_o = 0
            for kt in range(nkt):
                if ems:
                    ems.pop(0)()
                o = max(0, kt * 128 - c * 512)
                diag = kt >= nkt - 4
                es = []
                for h in (0, 1):
                    h_sl = slice(64 * h, 64 * h + 64)
                    sp = s_ps.tile([128, 512], F32, tag=f"s{h}",
                                   name=f"s{h}_{c}_{kt}")
                    nc.tensor.matmul(sp[:, o:512],
                                     kt_sb[h_sl, kt * 128:(kt + 1) * 128],
                                     qt_sb[h_sl, c * 512 + o:(c + 1) * 512],
                                     start=True, stop=True)
                    esb = e_pool.tile([128, 512], BF16, tag=f"e{h}",
                                      name=f"e{h}_{c}_{kt}")
                    if h == 0:
                        nc.scalar.activation(out=esb[:, o:512],
                                             in_=sp[:, o:512], func=EXP)
                    else:
                        # Schraudolph exp on DVE: bf16 bits via int16 output
                        nc.vector.tensor_scalar(
                            out=esb[:, o:512].bitcast(I16), in0=sp[:, o:512],
                            scalar1=A16, scalar2=B16, op0=MULT, op1=ADD)
                    if diag:
                        nc.vector.tensor_mul(esb[:, o:o + 128],
                                             esb[:, o:o + 128], trimask[:])
                    es.append(esb)
                if prev is not None:
                    emit_pv(kt - 1, prev, prev_o)
                prev, prev_o = es, o
            emit_pv(nkt - 1, prev, prev_o)
            while ems:
                ems.pop(0)()

            # epilogue: denominator broadcast + reciprocal + normalize
            for h in (0, 1):
                r1 = rb_pool.tile([1, 512], F32, tag="r1", name=f"r1{h}_{c}")
                nc.vector.tensor_copy(r1[:], pv[h][64:65, :])
                rb = rb_pool.tile([128, 512], F32, tag="rb", name=f"rb{h}_{c}")
                nc.gpsimd.partition_broadcast(rb[:], r1[:], channels=128)
                nc.vector.reciprocal_approx_fast(out=rb[:], in_=rb[:])
                nc.vector.tensor_mul(at_sb[64 * h:64 * h + 64, c_sl],
                                     pv[h][0:64, :], rb[64 * h:64 * h + 64, :])
            nc.sync.dma_start(at_d.ap()[:, c_sl], at_sb[:, c_sl])

        # software pipeline: chunk c's QKV work-groups are interleaved into
        # the k-steps of attention on chunk c-1.
        for em in qkv_emitters(0):
            em()
        for c in range(1, NQ):
            do_attention(c - 1, qkv_emitters(c))
        do_attention(NQ - 1, [])

    nc.compile()
    return nc


def build_kernel_b(T=T_FULL, ln_affine=False):
    """Per core: slice of T/8 tokens: out-proj + residual(+bout folded on host
    into xb) + LayerNorm (gamma/beta applied only if ln_affine)."""
    nc = bacc.Bacc("TRN2", target_bir_lowering=False, debug=False)
    Tc = T // NCORES
    KD = D // 128
    IDENT = mybir.ActivationFunctionType.Identity

    at_d = nc.dram_tensor("at", [128, KD, Tc], BF16, kind="ExternalInput")
    wo_d = nc.dram_tensor("wout", [128, 2, KD, 512], BF16, kind="ExternalInput")
    xb_d = nc.dram_tensor("xb", [Tc, D], BF16, kind="ExternalInput")
    g_d = nc.dram_tensor("gamma", [128, D], F32, kind="ExternalInput")
    be_d = nc.dram_tensor("beta", [128, D], F32, kind="ExternalInput")
    y_d = nc.dram_tensor("y", [Tc, D], F32, kind="ExternalOutput")

    with tile.TileContext(nc) as tc, ExitStack() as ctx:
        const = ctx.enter_context(tc.tile_pool(name="const", bufs=1))
        work = ctx.enter_context(tc.tile_pool(name="work", bufs=4))
        stats = ctx.enter_context(tc.tile_pool(name="stats", bufs=8))
        ps = ctx.enter_context(tc.tile_pool(name="ps", bufs=4, space="PSUM"))

        # interleave at / wout-half0 DMAs so the first matmuls start early
        at_sb = const.tile([128, KD, Tc], BF16, tag="at")
        wo_sb = const.tile([128, 2, KD, 512], BF16, tag="wo")
        nc.sync.dma_start(at_sb[:, 0:4], at_d.ap()[:, 0:4])
        nc.sync.dma_start(wo_sb[:, 0, 0:4], wo_d.ap()[:, 0, 0:4])
        nc.sync.dma_start(at_sb[:, 4:8], at_d.ap()[:, 4:8])
        nc.sync.dma_start(wo_sb[:, 0, 4:8], wo_d.ap()[:, 0, 4:8])
        nc.sync.dma_start(wo_sb[:, 1], wo_d.ap()[:, 1])
        xb_tiles = []
        for tt in range(Tc // 128):
            xb_t = work.tile([128, D], BF16, tag="xb", name=f"xb_{tt}")
            nc.sync.dma_start(xb_t[:], xb_d.ap()[tt * 128:(tt + 1) * 128, :])
            xb_tiles.append(xb_t)
        if ln_affine:
            gam_b = const.tile([128, D], F32, tag="gam")
            bet_b = const.tile([128, D], F32, tag="bet")
            nc.sync.dma_start(gam_b[:], g_d.ap())
            nc.sync.dma_start(bet_b[:], be_d.ap())
        eps_sb = const.tile([128, 1], F32, tag="eps")
        nc.vector.memset(eps_sb[:], LN_EPS)

        for tt in range(Tc // 128):
            t_sl = slice(tt * 128, (tt + 1) * 128)
            xb_t = xb_tiles[tt]
            y_t = work.tile([128, D], F32, tag="y")
            for j in (0, 1):
                pp = ps.tile([128, 512], F32, tag="pp")
                for kt in range(KD):
                    nc.tensor.matmul(pp[:], at_sb[:, kt, t_sl],
                                     wo_sb[:, j, kt, :],
                                     start=(kt == 0), stop=(kt == KD - 1))
                nc.vector.tensor_add(y_t[:, j * 512:(j + 1) * 512], pp[:],
                                     xb_t[:, j * 512:(j + 1) * 512])
            st = stats.tile([128, 2, 6], F32, tag="st")
            nc.vector.bn_stats(st[:, 0, :], y_t[:, 0:512])
            nc.vector.bn_stats(st[:, 1, :], y_t[:, 512:1024])
            mv = stats.tile([128, 2], F32, tag="mv")
            nc.vector.bn_aggr(mv[:], st[:])
            sq = stats.tile([128, 1], F32, tag="sq")
            nc.scalar.activation(out=sq[:], in_=mv[:, 1:2], func=SQRT,
                                 bias=eps_sb[:], scale=1.0)
            rstd = stats.tile([128, 1], F32, tag="rstd")
            nc.vector.reciprocal(rstd[:], sq[:])
            # nm = -mu * rstd;  y = y * rstd + nm   (one ACT op)
            nm = stats.tile([128, 1], F32, tag="nm")
            nc.vector.tensor_scalar(out=nm[:], in0=mv[:, 0:1],
                                    scalar1=rstd[:], scalar2=-1.0,
                                    op0=MULT, op1=MULT)
            nc.scalar.activation(out=y_t[:], in_=y_t[:], func=IDENT,
                                 bias=nm[:], scale=rstd[:])
            if ln_affine:
                nc.vector.tensor_mul(y_t[:], y_t[:], gam_b[:])
                nc.vector.tensor_add(y_t[:], y_t[:], bet_b[:])
            nc.sync.dma_start(y_d.ap()[t_sl, :], y_t[:])

    nc.compile()
    return nc


def _get_kernels(T=T_FULL, ln_affine=False):
    key = (T, ln_affine)
    if key not in _CACHE:
        _CACHE[key] = (build_kernel_a(T), build_kernel_b(T, ln_affine))
    return _CACHE[key]


def kernel(x, Wqkv, bqkv, Wout, bout, gamma, beta):
    x = np.asarray(x, dtype=np.float32)
    Wqkv = np.asarray(Wqkv, dtype=np.float32)
    bqkv = np.asarray(bqkv, dtype=np.float32)
    Wout = np.asarray(Wout, dtype=np.float32)
    bout = np.asarray(bout, dtype=np.float32)
    gamma = np.asarray(gamma, dtype=np.float32)
    beta = np.asarray(beta, dtype=np.float32)

    B, T, D_ = x.shape
    assert B == 1 and D_ == D
    d = D // HEADS
    scale = d ** -0.5
    x2d = np.ascontiguousarray(x[0])
    KD = D // 128
    NQ = T // 512

    # host-side layout prep (free): xt[p, c, k, j] = x[c*512+j, k*128+p]
    xt = np.ascontiguousarray(
        x2d.T.reshape(KD, 128, NQ, 512).transpose(1, 2, 0, 3)).astype(BF16NP)
    trimask = np.triu(np.ones((128, 128), np.float32)).astype(BF16NP)

    ln_affine = not (np.all(gamma == 1.0) and np.all(beta == 0.0))
    nc_a, nc_b = _get_kernels(T, ln_affine)

    in_maps_a = []
    for c in range(NCORES):
        r = slice(c * 128, (c + 1) * 128)
        wq = Wqkv[0 * D:1 * D][r]            # [128, D]
        wk = Wqkv[1 * D:2 * D][r] * scale
        wv = Wqkv[2 * D:3 * D][r]
        bv = bqkv[2 * D:3 * D][r]
        # stationary layout [128 part=D-slice, kt, 128 out]
        wpack = np.stack([
            w.T.reshape(KD, 128, 128).transpose(1, 0, 2)
            for w in (wq, wk, wv)], axis=1)  # [128, 3, KD, 128]
        bpack = np.concatenate([
            bqkv[0 * D:1 * D][r].reshape(128, 1),
            (bqkv[1 * D:2 * D][r] * scale).reshape(128, 1),
            np.tile(bv.reshape(1, 128), (128, 1)),
        ], axis=1)  # [128, 130]
        in_maps_a.append({
            "xt": xt,
            "trimask": trimask,
            "wpack": np.ascontiguousarray(wpack).astype(BF16NP),
            "bpack": np.ascontiguousarray(bpack),
        })
    res_a = run_bass_kernel_spmd(nc_a, in_maps_a, core_ids=list(range(NCORES)))
    LAST_RESULTS["a"] = res_a
    at_full = np.concatenate([res_a.results[c]["at_out"] for c in range(NCORES)],
                             axis=0)  # [D, T] bf16

    Tc = T // NCORES
    wout_st = np.ascontiguousarray(
        Wout.T.reshape(KD, 128, 2, 512).transpose(1, 2, 0, 3)).astype(BF16NP)
    gam_rep = np.ascontiguousarray(np.tile(gamma.reshape(1, D), (128, 1)))
    bet_rep = np.ascontiguousarray(np.tile(beta.reshape(1, D), (128, 1)))
    in_maps_b = []
    for c in range(NCORES):
        t_sl = slice(c * Tc, (c + 1) * Tc)
        at_c = at_full[:, t_sl]  # [D, Tc] bf16
        in_maps_b.append({
            "at": np.ascontiguousarray(at_c.reshape(KD, 128, Tc).transpose(1, 0, 2)),
            "wout": wout_st,
            "xb": np.ascontiguousarray(x2d[t_sl] + bout[None, :]).astype(BF16NP),
            "gamma": gam_rep,
            "beta": bet_rep,
        })
    res_b = run_bass_kernel_spmd(nc_b, in_maps_b, core_ids=list(range(NCORES)))
    LAST_RESULTS["b"] = res_b
    y = np.concatenate([res_b.results[c]["y"] for c in range(NCORES)], axis=0)
    return y.reshape(1, T, D).astype(np.float32)
